# revision 29
# baseline (speedup 1.0000x reference)
"""Multi-head causal+padded attention on 8 Trainium2 NeuronCores.

Sharding: core c handles batch b = c//2 and head-group g = c%2 (8 of 16 heads).
Each core computes its q/k/v projections (512 output dims) and attention for
its 8 heads over the full 2048-seq, producing out^T [512, 2048]; the host
transposes/concats into the full [4, 2048, 1024] output.

Device algorithm (per core), all matmul inputs bf16 (PE 1 cycle/col warm):
  xT [1024,2048] bf16 resident in SBUF; qT/kT = W^T-slices @ xT (output layout
  [outdim, seq], bf16); v in natural [seq, outdim] layout, bias-added,
  pad-masked, bf16, augmented with a 65th column = pad mask + 1e-30 (the eps
  makes softmax denominators strictly positive so no separate guard is
  needed).
  Scores are computed transposed (sT[k,q] = k_h^T q_h) per 128-k-block in
  PAIRS sharing one [128,1024] 2-bank PSUM tile, exp'd in a single scalar-
  engine activation (scale=1/8 folded in), causal-masked on diagonal blocks,
  then att^T @ [v|pad] accumulates in PSUM giving out^T[d,q] plus the softmax
  denominator (row 64) in one chain. The attention stream is software-
  pipelined one step: scores for block-pair i issue before the AV matmuls of
  pair i-1, so the tensor engine never waits on the scalar engine and the
  HAM clock gate stays warm (2.4 GHz).
  Normalization: reciprocal_approx_fast of the denominator row, times the
  q-pad row, partition-broadcast and applied on the gpsimd engine.
"""
import os
import sys

sys.path.insert(0, "/opt/trn_rl_repo")

import numpy as np

S = 2048
E = 1024
D = 64
H = 16          # total heads
HPC = 8         # heads per core
OC = HPC * D    # 512 output dims per core
EB = E // 128   # 8 contraction blocks
NSB = S // 128  # 16 seq blocks
NCH = S // 512  # 4 q-chunks
B = 4
NCORES = 8

_cache = {}


def _build_nc():
    from concourse import bacc
    import concourse.tile as tile
    import concourse.mybir as mybir

    F32 = mybir.dt.float32
    BF16 = mybir.dt.bfloat16
    AF = mybir.ActivationFunctionType

    nc = bacc.Bacc("TRN2", target_bir_lowering=False, debug=False,
                   num_devices=NCORES)
    xT = nc.dram_tensor("xT", [E, S], BF16, kind="ExternalInput").ap()
    wqT = nc.dram_tensor("wqT", [E, OC], BF16, kind="ExternalInput").ap()
    wkT = nc.dram_tensor("wkT", [E, OC], BF16, kind="ExternalInput").ap()
    wvT = nc.dram_tensor("wvT", [E, OC], BF16, kind="ExternalInput").ap()
    bq = nc.dram_tensor("bq", [OC], F32, kind="ExternalInput").ap()
    bk = nc.dram_tensor("bk", [OC], F32, kind="ExternalInput").ap()
    bv = nc.dram_tensor("bv", [OC], F32, kind="ExternalInput").ap()
    pad = nc.dram_tensor("pad", [S], F32, kind="ExternalInput").ap()
    outT = nc.dram_tensor("outT", [OC, S], F32, kind="ExternalOutput").ap()

    with tile.TileContext(nc) as tc:
        with tc.tile_pool(name="const", bufs=1) as cpool, \
             tc.tile_pool(name="big", bufs=1) as bigpool:

            # ---------------- constants ----------------
            pad_sb = cpool.tile([128, NSB], F32, tag="pad_f")
            nc.sync.dma_start(pad_sb[:], pad.rearrange("(b p) -> p b", p=128))
            pad_row = cpool.tile([1, S], F32, tag="padr_f")
            nc.sync.dma_start(pad_row[:], pad.rearrange("(a s) -> a s", a=1))

            bq_sb = cpool.tile([128, 4], F32, tag="bq")
            nc.sync.dma_start(bq_sb[:], bq.rearrange("(b p) -> p b", p=128))
            bk_sb = cpool.tile([128, 4], F32, tag="bk")
            nc.sync.dma_start(bk_sb[:], bk.rearrange("(b p) -> p b", p=128))
            bv_row = cpool.tile([1, OC], F32, tag="bv_row")
            nc.sync.dma_start(bv_row[:], bv.rearrange("(a c) -> a c", a=1))
            bv_tile = cpool.tile([128, OC], F32, tag="bv_tile")
            nc.gpsimd.partition_broadcast(bv_tile[:], bv_row[:])

            # tri[k, q] = 1 where k <= q else 0 (local 128x128 diagonal block)
            tri = cpool.tile([128, 128], BF16, tag="tri")
            nc.gpsimd.memset(tri[:], 1.0)
            nc.gpsimd.affine_select(
                out=tri[:], in_=tri[:], compare_op=mybir.AluOpType.is_ge,
                fill=0.0, base=0, pattern=[[1, 128]], channel_multiplier=-1)

            qT_sb = bigpool.tile([128, 4 * S], BF16, tag="qT")
            # kT stored twice, zero-padded by head parity, so score matmuls
            # contract over the full 128 partitions (keeps the HAM clock
            # gate seeing a fully-active PE array): kTz[0] holds even heads
            # in rows 0:64 (rows 64:128 zero), kTz[1] odd heads in 64:128.
            kTz = [bigpool.tile([128, 4 * S], BF16, tag=f"kTz{z}",
                                name=f"kTz{z}") for z in range(2)]
            # v padded to 128 cols per (block, head): cols 0:64 = v, col 64 =
            # pad+eps (softmax denominator), cols 65:128 zero.
            v_aug = bigpool.tile([128, NSB * HPC * 128], BF16, tag="v_aug")
            v_r = v_aug[:].rearrange("p (b h c) -> p b h c", b=NSB, h=HPC)

            # ======== phase 1: projections ========
            with tc.tile_pool(name="xw", bufs=6) as xw, \
                 tc.tile_pool(name="xp", bufs=1) as xp, \
                 tc.tile_pool(name="psP", bufs=4, space="PSUM") as psP:

                def load_w(wdram, nm):
                    halves = []
                    for half in range(2):
                        w_sb = xw.tile([128, 4 * OC], BF16, tag="w",
                                       name=f"w_{nm}{half}")
                        for i in range(4):
                            eb = half * 4 + i
                            nc.sync.dma_start(
                                w_sb[:, i * OC:(i + 1) * OC],
                                wdram[eb * 128:(eb + 1) * 128, :])
                        halves.append(w_sb)
                    return halves

                # DMA order: wq first (q-proj can start early), then x in
                # seq-chunk-major order, then wk/wv (needed later).
                wq_h = load_w(wqT, "q")
                x_sb = xp.tile([128, EB * S], BF16, tag="x_sb")
                for scn in range(4):
                    for eb in range(EB):
                        nc.sync.dma_start(
                            x_sb[:, eb * S + scn * 512:
                                 eb * S + (scn + 1) * 512],
                            xT[eb * 128:(eb + 1) * 128,
                               scn * 512:(scn + 1) * 512])
                wk_h = load_w(wkT, "k")
                wv_h = load_w(wvT, "v")

                # zero fillers: v_aug cols 64:128 (col 64 then set to 1),
                # and the dead parity halves of the kTz copies
                nc.vector.memset(v_aug[:], 0.0)
                nc.vector.memset(kTz[0][64:128, :], 0.0)
                nc.vector.memset(kTz[1][0:64, :], 0.0)
                nc.gpsimd.memset(v_r[:, :, :, 64], 1.0)

                # q/k projections (out layout [o, s])
                for wh, bias_sb, which in ((wq_h, bq_sb, "q"),
                                           (wk_h, bk_sb, "k")):
                    for scn in range(4):
                        for ob in range(4):
                            ps = psP.tile([128, 512], F32, tag="ps_proj")
                            for eb in range(EB):
                                w_sb = wh[eb // 4]
                                i = eb % 4
                                nc.tensor.matmul(
                                    ps[:],
                                    w_sb[:, i * OC + ob * 128:
                                         i * OC + (ob + 1) * 128],
                                    x_sb[:, eb * S + scn * 512:
                                         eb * S + (scn + 1) * 512],
                                    start=(eb == 0), stop=(eb == EB - 1))
                            cols = slice(ob * S + scn * 512,
                                         ob * S + (scn + 1) * 512)
                            if which == "q":
                                nc.vector.tensor_scalar_add(
                                    qT_sb[:, cols], ps[:],
                                    bias_sb[:, ob:ob + 1])
                            else:
                                nc.vector.tensor_scalar_add(
                                    kTz[0][0:64, cols], ps[0:64, :],
                                    bias_sb[0:64, ob:ob + 1])
                                nc.vector.tensor_scalar_add(
                                    kTz[1][64:128, cols], ps[64:128, :],
                                    bias_sb[64:128, ob:ob + 1])

                # v projection (natural [s, o] layout, bias+pad, bf16)
                for sb in range(NSB):
                    ps = psP.tile([128, 512], F32, tag="ps_proj")
                    for eb in range(EB):
                        w_sb = wv_h[eb // 4]
                        i = eb % 4
                        nc.tensor.matmul(
                            ps[:],
                            x_sb[:, eb * S + sb * 128:eb * S + (sb + 1) * 128],
                            w_sb[:, i * OC:(i + 1) * OC],
                            start=(eb == 0), stop=(eb == EB - 1))
                    nc.vector.tensor_add(
                        v_r[:, sb, :, 0:64],
                        ps[:].rearrange("p (h c) -> p h c", h=HPC),
                        bv_tile[:].rearrange("p (h c) -> p h c", h=HPC))
                    nc.vector.tensor_scalar_mul(
                        v_r[:, sb, :, 0:65], v_r[:, sb, :, 0:65],
                        pad_sb[:, sb:sb + 1])
                # denominator epsilon: 65th columns become pad + 1e-30 so the
                # softmax denominator is strictly positive even for fully
                # masked rows (output still exact 0 since v rows are zeroed)
                nc.vector.tensor_scalar_add(v_r[:, :, :, 64],
                                            v_r[:, :, :, 64], 1e-14)

            stage = os.environ.get("MHA_STAGE", "full")
            if stage == "proj":
                with tc.tile_pool(name="dbg", bufs=2) as dbg:
                    qdump = dbg.tile([128, S], F32, tag="qd", name="qdump")
                    nc.vector.tensor_copy(qdump[:], qT_sb[:, 0:S])
                    nc.sync.dma_start(outT[0:128, :], qdump[:])
                    kdump = dbg.tile([128, S], F32, tag="qd", name="kdump")
                    nc.vector.tensor_copy(kdump[0:64, :], kTz[0][0:64, 0:S])
                    nc.vector.tensor_copy(kdump[64:128, :],
                                          kTz[1][64:128, 0:S])
                    nc.sync.dma_start(outT[128:256, :], kdump[:])
                    vdump = dbg.tile([128, 1024], F32, tag="vd", name="vdump")
                    nc.vector.tensor_copy(vdump[:], v_aug[:, 0:1024])
                    nc.sync.dma_start(outT[256:384, 0:1024], vdump[:])

            # ======== phase 2: attention (software-pipelined) ========
            with tc.tile_pool(name="attp", bufs=4) as attp, \
                 tc.tile_pool(name="work", bufs=4) as work, \
                 tc.tile_pool(name="outp", bufs=3) as outp, \
                 tc.tile_pool(name="psS", bufs=2, space="PSUM") as psS, \
                 tc.tile_pool(name="psAv", bufs=2, space="PSUM") as psAv:

                avs = {}

                def issue_av(item):
                    """AV matmuls for a finished score pair; on the last pair
                    of a head, chain that head's normalization + output."""
                    scn, hp, p, i, att_t = item
                    q0 = scn * 512
                    nkb = 4 * scn + 4
                    h = 2 * hp + i
                    av = avs[(scn, hp, i)]
                    for half in range(2):
                        kb = 2 * p + half
                        lstart = max(0, kb * 128 - q0)
                        nc.tensor.matmul(
                            av[:, lstart:512],
                            v_r[:, kb, h, :],
                            att_t[:, half * 512 + lstart:(half + 1) * 512],
                            start=(kb == 0), stop=(kb == nkb - 1))
                    if p != nkb // 2 - 1:
                        return
                    if stage == "av":
                        if (scn, hp, i) in ((0, 0, 0), (1, 0, 0)):
                            row = 0 if scn == 0 else 65
                            o_dbg = outp.tile([65, 512], F32, tag="osb",
                                              name="o_dbg")
                            nc.vector.tensor_copy(o_dbg[:], av[0:65, :])
                            nc.sync.dma_start(
                                outT[row:row + 65, q0:q0 + 512], o_dbg[:])
                        return
                    # normalization for head h of chunk scn. Kept off the
                    # DVE FIFO where possible: the denominator row comes out
                    # of PSUM via DMA, the pad multiply runs on gpsimd, so
                    # only the reciprocal and the final scale occupy DVE
                    # (long DVE chains here would delay the tri-masks queued
                    # behind them and stall the AV matmuls).
                    r0 = work.tile([1, 512], F32, tag="rt", name="r0")
                    nc.vector.tensor_copy(r0[:], av[64:65, :])
                    r1 = work.tile([1, 512], F32, tag="rt", name="r1")
                    nc.vector.reciprocal_approx_fast(out=r1[:], in_=r0[:])
                    r2 = work.tile([1, 512], F32, tag="rt", name="r2")
                    nc.gpsimd.tensor_mul(r2[:], r1[:],
                                         pad_row[:, q0:q0 + 512])
                    bc = work.tile([64, 512], F32, tag="bc", name="bc")
                    nc.gpsimd.partition_broadcast(bc[:], r2[:])
                    o_sb = outp.tile([64, 512], F32, tag="osb", name="o_sb")
                    nc.vector.tensor_mul(o_sb[:], av[0:64, :], bc[:])
                    nc.sync.dma_start(
                        outT[h * 64:(h + 1) * 64, q0:q0 + 512], o_sb[:])

                pend = None
                for scn in range(NCH if stage != "proj" else 0):
                    q0 = scn * 512
                    npairs = 2 * scn + 2
                    for hp in range(4):
                        for i in range(2):
                            avs[(scn, hp, i)] = psAv.tile(
                                [128, 512], F32, tag=f"ps_av{i}",
                                name=f"ps_av{i}")
                        for p in range(npairs):
                            for i in range(2):
                                h = 2 * hp + i
                                ob = h // 2
                                ssb = psS.tile([128, 1024], F32, tag="ps_s")
                                att_t = attp.tile([128, 1024], BF16,
                                                  tag="att")
                                for half in range(2):
                                    kb = 2 * p + half
                                    nc.tensor.matmul(
                                        ssb[:, half * 512:(half + 1) * 512],
                                        kTz[h % 2][:,
                                                   ob * S + kb * 128:
                                                   ob * S + (kb + 1) * 128],
                                        qT_sb[:, ob * S + q0:
                                              ob * S + q0 + 512],
                                        start=True, stop=True)
                                nc.scalar.activation(att_t[:], ssb[:],
                                                     AF.Exp, scale=0.125)
                                for half in range(2):
                                    kb = 2 * p + half
                                    if kb >= 4 * scn:
                                        off = half * 512 + (kb * 128 - q0)
                                        nc.vector.tensor_mul(
                                            att_t[:, off:off + 128],
                                            att_t[:, off:off + 128], tri[:])
                                if stage == "att" and \
                                        (scn, hp, p, i) in ((0, 0, 0, 0),
                                                            (1, 0, 0, 0)):
                                    row = 0 if scn == 0 else 128
                                    a_dbg = attp.tile([128, 1024], F32,
                                                      tag="adbg",
                                                      name="a_dbg")
                                    nc.vector.tensor_copy(a_dbg[:], att_t[:])
                                    nc.sync.dma_start(
                                        outT[row:row + 128, 0:1024], a_dbg[:])
                                if stage != "att":
                                    if pend is not None:
                                        issue_av(pend)
                                    pend = (scn, hp, p, i, att_t)
                if pend is not None:
                    issue_av(pend)
    nc.compile()
    return nc


def get_nc():
    key = os.environ.get("MHA_STAGE", "full")
    if key not in _cache:
        _cache[key] = _build_nc()
    return _cache[key]


def make_in_maps(input_x, pad_mask, Wq, bq, Wk, bk, Wv, bv):
    import ml_dtypes

    BF = ml_dtypes.bfloat16
    input_x = np.asarray(input_x, dtype=np.float32)
    pad_f = np.asarray(pad_mask).astype(np.float32)
    Wq = np.asarray(Wq, dtype=np.float32)
    Wk = np.asarray(Wk, dtype=np.float32)
    Wv = np.asarray(Wv, dtype=np.float32)
    bq = np.asarray(bq, dtype=np.float32)
    bk = np.asarray(bk, dtype=np.float32)
    bv = np.asarray(bv, dtype=np.float32)

    xTs = [np.ascontiguousarray(input_x[b].T).astype(BF) for b in range(B)]
    wslices = {}
    for g in range(2):
        sl = slice(g * OC, (g + 1) * OC)
        wslices[g] = (np.ascontiguousarray(Wq[sl].T).astype(BF),
                      np.ascontiguousarray(Wk[sl].T).astype(BF),
                      np.ascontiguousarray(Wv[sl].T).astype(BF),
                      np.ascontiguousarray(bq[sl]),
                      np.ascontiguousarray(bk[sl]),
                      np.ascontiguousarray(bv[sl]))
    in_maps = []
    for c in range(NCORES):
        b, g = c // 2, c % 2
        wq_t, wk_t, wv_t, bq_s, bk_s, bv_s = wslices[g]
        in_maps.append({
            "xT": xTs[b], "wqT": wq_t, "wkT": wk_t, "wvT": wv_t,
            "bq": bq_s, "bk": bk_s, "bv": bv_s,
            "pad": np.ascontiguousarray(pad_f[b]),
        })
    return in_maps


def assemble(results):
    out = np.empty((B, S, E), dtype=np.float32)
    for c in range(NCORES):
        b, g = c // 2, c % 2
        out[b, :, g * OC:(g + 1) * OC] = results[c]["outT"].T
    return out


def kernel(input_x, pad_mask, Wq, bq, Wk, bk, Wv, bv):
    from concourse.bass_utils import run_bass_kernel_spmd
    nc = get_nc()
    in_maps = make_in_maps(input_x, pad_mask, Wq, bq, Wk, bk, Wv, bv)
    res = run_bass_kernel_spmd(nc, in_maps, core_ids=list(range(NCORES)))
    if res.exec_time_ns is not None:
        print(f"HW exec time: {res.exec_time_ns} ns")
    return assemble(res.results)


# revision 30
# speedup vs baseline: 1.9466x; 1.9466x over previous
"""Multi-head causal+padded attention on 8 Trainium2 NeuronCores.

Sharding: core c handles batch b = c//2 and head-group g = c%2 (8 of 16 heads).
Each core computes its q/k/v projections (512 output dims) and attention for
its 8 heads over the full 2048-seq, producing out^T [512, 2048]; the host
transposes/concats into the full [4, 2048, 1024] output.

Device algorithm (per core), all matmul inputs bf16 (PE 1 cycle/col warm):
  xT [1024,2048] bf16 resident in SBUF; qT/kT = W^T-slices @ xT (output layout
  [outdim, seq], bf16); v in natural [seq, outdim] layout, bias-added,
  pad-masked, bf16, augmented with a 65th column = pad mask + 1e-30 (the eps
  makes softmax denominators strictly positive so no separate guard is
  needed).
  Scores are computed transposed (sT[k,q] = k_h^T q_h) per 128-k-block in
  PAIRS sharing one [128,1024] 2-bank PSUM tile, exp'd in a single scalar-
  engine activation (scale=1/8 folded in), causal-masked on diagonal blocks,
  then att^T @ [v|pad] accumulates in PSUM giving out^T[d,q] plus the softmax
  denominator (row 64) in one chain. The attention stream is software-
  pipelined one step: scores for block-pair i issue before the AV matmuls of
  pair i-1, so the tensor engine never waits on the scalar engine and the
  HAM clock gate stays warm (2.4 GHz).
  Normalization: reciprocal_approx_fast of the denominator row, times the
  q-pad row, partition-broadcast and applied on the gpsimd engine.
"""
import os
import sys

sys.path.insert(0, "/opt/trn_rl_repo")

import numpy as np

S = 2048
E = 1024
D = 64
H = 16          # total heads
HPC = 8         # heads per core
OC = HPC * D    # 512 output dims per core
EB = E // 128   # 8 contraction blocks
NSB = S // 128  # 16 seq blocks
NCH = S // 512  # 4 q-chunks
B = 4
NCORES = 8

_cache = {}


def _build_nc():
    from concourse import bacc
    import concourse.tile as tile
    import concourse.mybir as mybir

    F32 = mybir.dt.float32
    BF16 = mybir.dt.bfloat16
    AF = mybir.ActivationFunctionType

    nc = bacc.Bacc("TRN2", target_bir_lowering=False, debug=False,
                   num_devices=NCORES)
    xT = nc.dram_tensor("xT", [E, S], BF16, kind="ExternalInput").ap()
    wqT = nc.dram_tensor("wqT", [E, OC], BF16, kind="ExternalInput").ap()
    wkT = nc.dram_tensor("wkT", [E, OC], BF16, kind="ExternalInput").ap()
    wvT = nc.dram_tensor("wvT", [E, OC], BF16, kind="ExternalInput").ap()
    bq = nc.dram_tensor("bq", [OC], F32, kind="ExternalInput").ap()
    bk = nc.dram_tensor("bk", [OC], F32, kind="ExternalInput").ap()
    bv = nc.dram_tensor("bv", [OC], F32, kind="ExternalInput").ap()
    pad = nc.dram_tensor("pad", [S], F32, kind="ExternalInput").ap()
    outT = nc.dram_tensor("outT", [OC, S], F32, kind="ExternalOutput").ap()

    with tile.TileContext(nc) as tc:
        with tc.tile_pool(name="const", bufs=1) as cpool, \
             tc.tile_pool(name="big", bufs=1) as bigpool:

            # ---------------- constants ----------------
            pad_sb = cpool.tile([128, NSB], F32, tag="pad_f")
            nc.sync.dma_start(pad_sb[:], pad.rearrange("(b p) -> p b", p=128))
            pad_row = cpool.tile([1, S], F32, tag="padr_f")
            nc.sync.dma_start(pad_row[:], pad.rearrange("(a s) -> a s", a=1))

            bq_sb = cpool.tile([128, 4], F32, tag="bq")
            nc.sync.dma_start(bq_sb[:], bq.rearrange("(b p) -> p b", p=128))
            bk_sb = cpool.tile([128, 4], F32, tag="bk")
            nc.sync.dma_start(bk_sb[:], bk.rearrange("(b p) -> p b", p=128))
            bv_row = cpool.tile([1, OC], F32, tag="bv_row")
            nc.sync.dma_start(bv_row[:], bv.rearrange("(a c) -> a c", a=1))
            bv_tile = cpool.tile([128, OC], F32, tag="bv_tile")
            nc.gpsimd.partition_broadcast(bv_tile[:], bv_row[:])

            # tri[k, q] = 1 where k <= q else 0 (local 128x128 diagonal block)
            tri = cpool.tile([128, 128], BF16, tag="tri")
            nc.gpsimd.memset(tri[:], 1.0)
            nc.gpsimd.affine_select(
                out=tri[:], in_=tri[:], compare_op=mybir.AluOpType.is_ge,
                fill=0.0, base=0, pattern=[[1, 128]], channel_multiplier=-1)

            qT_sb = bigpool.tile([128, 4 * S], BF16, tag="qT")
            # kT stored twice, zero-padded by head parity, so score matmuls
            # contract over the full 128 partitions (keeps the HAM clock
            # gate seeing a fully-active PE array): kTz[0] holds even heads
            # in rows 0:64 (rows 64:128 zero), kTz[1] odd heads in 64:128.
            kTz = [bigpool.tile([128, 4 * S], BF16, tag=f"kTz{z}",
                                name=f"kTz{z}") for z in range(2)]
            # v padded to 128 cols per (block, head): cols 0:64 = v, col 64 =
            # pad+eps (softmax denominator), cols 65:128 zero.
            v_aug = bigpool.tile([128, NSB * HPC * 128], BF16, tag="v_aug")
            v_r = v_aug[:].rearrange("p (b h c) -> p b h c", b=NSB, h=HPC)

            # ======== phase 1: projections ========
            with tc.tile_pool(name="xw", bufs=6) as xw, \
                 tc.tile_pool(name="xp", bufs=1) as xp, \
                 tc.tile_pool(name="psP", bufs=4, space="PSUM") as psP:

                def load_w(wdram, nm):
                    halves = []
                    for half in range(2):
                        w_sb = xw.tile([128, 4 * OC], BF16, tag="w",
                                       name=f"w_{nm}{half}")
                        for i in range(4):
                            eb = half * 4 + i
                            nc.sync.dma_start(
                                w_sb[:, i * OC:(i + 1) * OC],
                                wdram[eb * 128:(eb + 1) * 128, :])
                        halves.append(w_sb)
                    return halves

                # DMA order: wq first (q-proj can start early), then x in
                # seq-chunk-major order, then wk/wv (needed later).
                wq_h = load_w(wqT, "q")
                x_sb = xp.tile([128, EB * S], BF16, tag="x_sb")
                for scn in range(4):
                    for eb in range(EB):
                        nc.sync.dma_start(
                            x_sb[:, eb * S + scn * 512:
                                 eb * S + (scn + 1) * 512],
                            xT[eb * 128:(eb + 1) * 128,
                               scn * 512:(scn + 1) * 512])
                wk_h = load_w(wkT, "k")
                wv_h = load_w(wvT, "v")

                # zero fillers: v_aug cols 64:128 (col 64 then set to 1),
                # and the dead parity halves of the kTz copies
                nc.vector.memset(v_aug[:], 0.0)
                nc.vector.memset(kTz[0][64:128, :], 0.0)
                nc.vector.memset(kTz[1][0:64, :], 0.0)
                nc.gpsimd.memset(v_r[:, :, :, 64], 1.0)

                # q/k projections (out layout [o, s])
                for wh, bias_sb, which in ((wq_h, bq_sb, "q"),
                                           (wk_h, bk_sb, "k")):
                    for scn in range(4):
                        for ob in range(4):
                            ps = psP.tile([128, 512], F32, tag="ps_proj")
                            for eb in range(EB):
                                w_sb = wh[eb // 4]
                                i = eb % 4
                                nc.tensor.matmul(
                                    ps[:],
                                    w_sb[:, i * OC + ob * 128:
                                         i * OC + (ob + 1) * 128],
                                    x_sb[:, eb * S + scn * 512:
                                         eb * S + (scn + 1) * 512],
                                    start=(eb == 0), stop=(eb == EB - 1))
                            cols = slice(ob * S + scn * 512,
                                         ob * S + (scn + 1) * 512)
                            if which == "q":
                                nc.vector.tensor_scalar_add(
                                    qT_sb[:, cols], ps[:],
                                    bias_sb[:, ob:ob + 1])
                            else:
                                nc.vector.tensor_scalar_add(
                                    kTz[0][0:64, cols], ps[0:64, :],
                                    bias_sb[0:64, ob:ob + 1])
                                nc.vector.tensor_scalar_add(
                                    kTz[1][64:128, cols], ps[64:128, :],
                                    bias_sb[64:128, ob:ob + 1])

                # v projection (natural [s, o] layout, bias+pad, bf16)
                for sb in range(NSB):
                    ps = psP.tile([128, 512], F32, tag="ps_proj")
                    for eb in range(EB):
                        w_sb = wv_h[eb // 4]
                        i = eb % 4
                        nc.tensor.matmul(
                            ps[:],
                            x_sb[:, eb * S + sb * 128:eb * S + (sb + 1) * 128],
                            w_sb[:, i * OC:(i + 1) * OC],
                            start=(eb == 0), stop=(eb == EB - 1))
                    nc.vector.tensor_add(
                        v_r[:, sb, :, 0:64],
                        ps[:].rearrange("p (h c) -> p h c", h=HPC),
                        bv_tile[:].rearrange("p (h c) -> p h c", h=HPC))
                    nc.vector.tensor_scalar_mul(
                        v_r[:, sb, :, 0:65], v_r[:, sb, :, 0:65],
                        pad_sb[:, sb:sb + 1])
                # denominator epsilon: 65th columns become pad + 1e-30 so the
                # softmax denominator is strictly positive even for fully
                # masked rows (output still exact 0 since v rows are zeroed)
                nc.vector.tensor_scalar_add(v_r[:, :, :, 64],
                                            v_r[:, :, :, 64], 1e-14)

            stage = os.environ.get("MHA_STAGE", "full")
            if stage == "proj":
                with tc.tile_pool(name="dbg", bufs=2) as dbg:
                    qdump = dbg.tile([128, S], F32, tag="qd", name="qdump")
                    nc.vector.tensor_copy(qdump[:], qT_sb[:, 0:S])
                    nc.sync.dma_start(outT[0:128, :], qdump[:])
                    kdump = dbg.tile([128, S], F32, tag="qd", name="kdump")
                    nc.vector.tensor_copy(kdump[0:64, :], kTz[0][0:64, 0:S])
                    nc.vector.tensor_copy(kdump[64:128, :],
                                          kTz[1][64:128, 0:S])
                    nc.sync.dma_start(outT[128:256, :], kdump[:])
                    vdump = dbg.tile([128, 1024], F32, tag="vd", name="vdump")
                    nc.vector.tensor_copy(vdump[:], v_aug[:, 0:1024])
                    nc.sync.dma_start(outT[256:384, 0:1024], vdump[:])

            # ======== phase 2: attention (software-pipelined) ========
            with tc.tile_pool(name="attp", bufs=4) as attp, \
                 tc.tile_pool(name="work", bufs=4) as work, \
                 tc.tile_pool(name="outp", bufs=3) as outp, \
                 tc.tile_pool(name="psS", bufs=2, space="PSUM") as psS, \
                 tc.tile_pool(name="psAv", bufs=2, space="PSUM") as psAv:

                avs = {}

                def issue_av(item):
                    """AV matmuls for a finished score pair; on the last pair
                    of a head, chain that head's normalization + output."""
                    scn, hp, p, i, att_t = item
                    q0 = scn * 512
                    nkb = 4 * scn + 4
                    h = 2 * hp + i
                    av = avs[(scn, hp, i)]
                    for half in range(2):
                        kb = 2 * p + half
                        lstart = max(0, kb * 128 - q0)
                        nc.tensor.matmul(
                            av[:, lstart:512],
                            v_r[:, kb, h, :],
                            att_t[:, half * 512 + lstart:(half + 1) * 512],
                            start=(kb == 0), stop=(kb == nkb - 1))
                    if p != nkb // 2 - 1:
                        return
                    if stage == "av":
                        if (scn, hp, i) in ((0, 0, 0), (1, 0, 0)):
                            row = 0 if scn == 0 else 65
                            o_dbg = outp.tile([65, 512], F32, tag="osb",
                                              name="o_dbg")
                            nc.vector.tensor_copy(o_dbg[:], av[0:65, :])
                            nc.sync.dma_start(
                                outT[row:row + 65, q0:q0 + 512], o_dbg[:])
                        return
                    # normalization for head h of chunk scn. Kept off the
                    # DVE FIFO where possible: the denominator row comes out
                    # of PSUM via DMA, the pad multiply runs on gpsimd, so
                    # only the reciprocal and the final scale occupy DVE
                    # (long DVE chains here would delay the tri-masks queued
                    # behind them and stall the AV matmuls).
                    r0 = work.tile([1, 512], F32, tag="rt", name="r0")
                    nc.vector.tensor_copy(r0[:], av[64:65, :])
                    r1 = work.tile([1, 512], F32, tag="rt", name="r1")
                    nc.vector.reciprocal_approx_fast(out=r1[:], in_=r0[:])
                    r2 = work.tile([1, 512], F32, tag="rt", name="r2")
                    nc.vector.tensor_mul(r2[:], r1[:],
                                         pad_row[:, q0:q0 + 512])
                    bc = work.tile([64, 512], F32, tag="bc", name="bc")
                    nc.gpsimd.partition_broadcast(bc[:], r2[:])
                    o_sb = outp.tile([64, 512], F32, tag="osb", name="o_sb")
                    nc.vector.tensor_mul(o_sb[:], av[0:64, :], bc[:])
                    nc.sync.dma_start(
                        outT[h * 64:(h + 1) * 64, q0:q0 + 512], o_sb[:])

                pend = None
                for scn in range(NCH if stage != "proj" else 0):
                    q0 = scn * 512
                    npairs = 2 * scn + 2
                    for hp in range(4):
                        for i in range(2):
                            avs[(scn, hp, i)] = psAv.tile(
                                [128, 512], F32, tag=f"ps_av{i}",
                                name=f"ps_av{i}")
                        for p in range(npairs):
                            for i in range(2):
                                h = 2 * hp + i
                                ob = h // 2
                                ssb = psS.tile([128, 1024], F32, tag="ps_s")
                                att_t = attp.tile([128, 1024], BF16,
                                                  tag="att")
                                for half in range(2):
                                    kb = 2 * p + half
                                    nc.tensor.matmul(
                                        ssb[:, half * 512:(half + 1) * 512],
                                        kTz[h % 2][:,
                                                   ob * S + kb * 128:
                                                   ob * S + (kb + 1) * 128],
                                        qT_sb[:, ob * S + q0:
                                              ob * S + q0 + 512],
                                        start=True, stop=True)
                                nc.scalar.activation(att_t[:], ssb[:],
                                                     AF.Exp, scale=0.125)
                                for half in range(2):
                                    kb = 2 * p + half
                                    if kb >= 4 * scn:
                                        off = half * 512 + (kb * 128 - q0)
                                        nc.vector.tensor_mul(
                                            att_t[:, off:off + 128],
                                            att_t[:, off:off + 128], tri[:])
                                if stage == "att" and \
                                        (scn, hp, p, i) in ((0, 0, 0, 0),
                                                            (1, 0, 0, 0)):
                                    row = 0 if scn == 0 else 128
                                    a_dbg = attp.tile([128, 1024], F32,
                                                      tag="adbg",
                                                      name="a_dbg")
                                    nc.vector.tensor_copy(a_dbg[:], att_t[:])
                                    nc.sync.dma_start(
                                        outT[row:row + 128, 0:1024], a_dbg[:])
                                if stage != "att":
                                    if pend is not None:
                                        issue_av(pend)
                                    pend = (scn, hp, p, i, att_t)
                if pend is not None:
                    issue_av(pend)
    nc.compile()
    return nc


def get_nc():
    key = os.environ.get("MHA_STAGE", "full")
    if key not in _cache:
        _cache[key] = _build_nc()
    return _cache[key]


def make_in_maps(input_x, pad_mask, Wq, bq, Wk, bk, Wv, bv):
    import ml_dtypes

    BF = ml_dtypes.bfloat16
    input_x = np.asarray(input_x, dtype=np.float32)
    pad_f = np.asarray(pad_mask).astype(np.float32)
    Wq = np.asarray(Wq, dtype=np.float32)
    Wk = np.asarray(Wk, dtype=np.float32)
    Wv = np.asarray(Wv, dtype=np.float32)
    bq = np.asarray(bq, dtype=np.float32)
    bk = np.asarray(bk, dtype=np.float32)
    bv = np.asarray(bv, dtype=np.float32)

    xTs = [np.ascontiguousarray(input_x[b].T).astype(BF) for b in range(B)]
    wslices = {}
    for g in range(2):
        sl = slice(g * OC, (g + 1) * OC)
        wslices[g] = (np.ascontiguousarray(Wq[sl].T).astype(BF),
                      np.ascontiguousarray(Wk[sl].T).astype(BF),
                      np.ascontiguousarray(Wv[sl].T).astype(BF),
                      np.ascontiguousarray(bq[sl]),
                      np.ascontiguousarray(bk[sl]),
                      np.ascontiguousarray(bv[sl]))
    in_maps = []
    for c in range(NCORES):
        b, g = c // 2, c % 2
        wq_t, wk_t, wv_t, bq_s, bk_s, bv_s = wslices[g]
        in_maps.append({
            "xT": xTs[b], "wqT": wq_t, "wkT": wk_t, "wvT": wv_t,
            "bq": bq_s, "bk": bk_s, "bv": bv_s,
            "pad": np.ascontiguousarray(pad_f[b]),
        })
    return in_maps


def assemble(results):
    out = np.empty((B, S, E), dtype=np.float32)
    for c in range(NCORES):
        b, g = c // 2, c % 2
        out[b, :, g * OC:(g + 1) * OC] = results[c]["outT"].T
    return out


def kernel(input_x, pad_mask, Wq, bq, Wk, bk, Wv, bv):
    from concourse.bass_utils import run_bass_kernel_spmd
    nc = get_nc()
    in_maps = make_in_maps(input_x, pad_mask, Wq, bq, Wk, bk, Wv, bv)
    res = run_bass_kernel_spmd(nc, in_maps, core_ids=list(range(NCORES)))
    if res.exec_time_ns is not None:
        print(f"HW exec time: {res.exec_time_ns} ns")
    return assemble(res.results)
